# revision 25
# baseline (speedup 1.0000x reference)
"""Trainium2 Bass kernel for nn_CrossDConv (sparse deformable attention conv).

Self-contained: host-side sharding/layout prep + Bass/Tile kernel, SPMD on
8 NeuronCores via run_bass_kernel_spmd.  Each core handles one
(batch, row-half) shard of the (4, 64, 64, 64) input.

All device work runs in a width-padded pixel space (66-wide rows, one zero
column each side, plus zero rows above/below the shard) so 3x3-conv taps
and bilinear-gather taps never wrap across rows: zero padding reproduces
the reference's conv zero-padding and zero-padded bilinear sampling
exactly, with no masks.

Math restructuring (exact, host-side):
  * Both depthwise3x3+pointwise1x1 offset branches and the modulation
    branch fuse into ONE composite 3x3 conv; outputs are packed as
    A = [ox; oy] (104 rows) and B = [u; u] (u duplicated, 104 rows) with
    u = scores - sparsity (softmax shift-invariance).  Biases are applied
    by the scalar engine (activation bias operand), not matmuls.
  * Unnormalized softmax weight e = exp(u)*sigmoid(u/tau), computed as
    e = (exp(u+bu)/2) * (1 + tanh(5*(u+bu))): tanh and exp share one
    activation table set, so the ACT engine never reloads tables.
  * Bilinear tent weights expanded over monomials {1, relu(t), -relu(-t)};
    the duplicated-row packing lets pairs of monomials stack into
    104-partition tiles, so the 3x3 recombination takes 5 G-matmuls
    (K=52/104) instead of 9; signs fold into static G matrices.
  * The 26th output row of the G-matmuls is the softmax denominator.
  * 1x1 "pc" conv commutes with the gather: the gather runs on
    y0 = pc_w @ x (computed directly pixel-major); pc bias folds into the
    first MLP bias, mlp bias 2 folds into the residual input.
  * Gather as banded matmul: normalized pixel-major A scattered into S^T
    (GPSIMD local_scatter, static indices), PE-transposed into q-major S
    chunks, PE matmuls against pixel-major y0.

The pipeline runs as 6 independent 384-pixel groups so Tile can overlap
phases across groups; all transposes use the PE (DMA-transpose costs
~1.2us of serial Sync-engine dispatch per call on this target).
"""

import numpy as np
import ml_dtypes

import concourse.bass as bass
import concourse.tile as tile
from concourse import mybir, library_config
from concourse.bass_utils import run_bass_kernel_spmd
from concourse.library_overlay import lower_extended_insts

BF16 = mybir.dt.bfloat16
F32 = mybir.dt.float32
F8 = mybir.dt.float8e4
I16 = mybir.dt.int16
W8SCALE = 256.0

# ------------------------------------------------------------------ geometry
B, C, H, W = 4, 64, 64, 64
OUTC = 64
N_CORES = 8
TAU = 0.1
NSAMP = 52
WP = W + 2                      # padded row width
ROWS_OUT = H // 2               # 32 output rows per core
LEAD = 63                       # leading zeros so P_OUT0 = 195 (=67+128)
SLAB_ROWS = 40                  # rows r0-2 .. r0+38 (zero-padded outside image)
P_SLAB = 2816                   # 63 + 40*66 + tail zeros, 22 chunks of 128
P_OUT0 = LEAD + 2 * WP          # 195
NP_OUT = ROWS_OUT * WP          # 2112 padded positions carrying outputs
NBLK = (NP_OUT + 127) // 128    # 17 pixel blocks
QSPAN = 512                     # q-window per block: [p0-67, p0+445)
NTAP = 25
NTAPD = 26
NTAPP = 32                      # padded tap stride
SCAT_BLKS = 3
NSCAT = (NBLK + SCAT_BLKS - 1) // SCAT_BLKS   # 6 groups
GCOLS = SCAT_BLKS * 128         # 384 pixels per group

# bf16 weight blob column layout
WB_IDENT = 0                    # [128, 128]
WB_GEV = 128                    # [52, 26]
WB_PCT = 154                    # [64, 64]
WB_W1T = 218
WB_W2T = 282
WB_G5 = 346                     # [116, 7*26]
WB_COLS = 528
# fp8 weight blob column layout
W8_CONV = 0                     # [128, 3*2*256] DR conv pairs (x W8SCALE)
W8_COLS = 1536

_CACHE = {}


# =====================================================================
# Device kernel
# =====================================================================

def _emit(nc, tc, d):
    from contextlib import ExitStack
    AF = mybir.ActivationFunctionType
    OP = mybir.AluOpType

    with ExitStack() as ctx:
        weights = ctx.enter_context(tc.tile_pool(name="weights", bufs=1))
        big = ctx.enter_context(tc.tile_pool(name="big", bufs=1))
        work = ctx.enter_context(tc.tile_pool(name="work", bufs=2))
        small = ctx.enter_context(tc.tile_pool(name="small", bufs=2))
        schunkp = ctx.enter_context(tc.tile_pool(name="schunk", bufs=3))
        psum = ctx.enter_context(tc.tile_pool(name="psum", bufs=1, space="PSUM"))
        psumA = ctx.enter_context(tc.tile_pool(name="psumA", bufs=2, space="PSUM"))
        psumT = ctx.enter_context(tc.tile_pool(name="psumT", bufs=1, space="PSUM"))

        nc.gpsimd.load_library(library_config.local_scatter)

        # ---------------- merged loads (conv inputs first)
        x8 = big.tile([128, P_SLAB], F8)
        nc.sync.dma_start(out=x8, in_=d["x8"][:, :])
        w8 = weights.tile([128, W8_COLS], F8)
        nc.sync.dma_start(out=w8, in_=d["wb8"][:, :])
        x2 = big.tile([128, P_SLAB], BF16)
        nc.sync.dma_start(out=x2, in_=d["x2"][:, :])
        wb = weights.tile([128, WB_COLS], BF16)
        nc.sync.dma_start(out=wb, in_=d["wb16"][:, :])
        wf = weights.tile([128, 4], F32)
        nc.sync.dma_start(out=wf, in_=d["wbf32"][:, :])
        sidx = weights.tile([128, NSCAT, SCAT_BLKS * NTAPP], I16)
        nc.sync.dma_start(out=sidx, in_=d["sidx"][:, :, :])
        xres = big.tile([C, NP_OUT], F32)
        nc.sync.dma_start(out=xres, in_=d["xres"][:, :])

        ident = wb[:, WB_IDENT : WB_IDENT + 128]
        gev = wb[0:52, WB_GEV : WB_GEV + 26]
        pcT = wb[0:C, WB_PCT : WB_PCT + 64]
        w1T = wb[0:OUTC, WB_W1T : WB_W1T + 64]
        w2T = wb[0:OUTC, WB_W2T : WB_W2T + 64]
        w8conv = w8[:, W8_CONV:W8_COLS].rearrange("p (t a m) -> p t a m",
                                                  t=3, a=2)
        g5 = wb[0:116, WB_G5 : WB_G5 + 182].rearrange("p (k t) -> p k t", k=7)
        bA = wf[0:116, 0:1]
        bT = wf[0:116, 1:2]
        bE = wf[0:116, 2:3]
        b1 = wf[0:OUTC, 3:4]

        def dr_window(base, gn):
            # [128, 2, gn] view of x8 with the two K-tiles WP apart
            sl = x8[:, base : base + gn]
            return bass.AP(tensor=sl.tensor, offset=sl.offset,
                           ap=[list(sl.ap[0]), [WP, 2], [1, gn]])

        # ---------------- y0 pixel-major (fp8), chunks 1..20 (used by gather)
        NQCH = P_SLAB // 128
        y0_pm = big.tile([128, NQCH, OUTC], F8)

        def emit_y0():
            for qg in range(5):
                accy = psum.tile([128, 4 * OUTC], F32, tag="ps_mm", bufs=2)
                for qi in range(4):
                    qc = 1 + qg * 4 + qi
                    s = qc * 128
                    nc.tensor.matmul(accy[:, qi * OUTC : (qi + 1) * OUTC],
                                     x2[0:C, s : s + 128], pcT, start=True,
                                     stop=True)
                if qg % 2 == 0:
                    nc.scalar.activation(y0_pm[:, 1 + qg * 4 : 5 + qg * 4, :],
                                         accy, AF.Copy)
                else:
                    nc.vector.tensor_copy(
                        y0_pm[:, 1 + qg * 4 : 5 + qg * 4, :], accy)

        # ---------------- main per-group pipeline
        DR = mybir.MatmulPerfMode.DoubleRow
        for grp in range(NSCAT):
            gs = grp * GCOLS
            ge = min(gs + GCOLS, NP_OUT)
            gn = ge - gs
            nblk_g = min(SCAT_BLKS, NBLK - grp * SCAT_BLKS)

            # ---- composite conv, fp8 DoubleRow: 3 matmuls per branch,
            #      K-tiles = (rows r-1/r+1, stacked) and (center row, WP away)
            accA = psum.tile([128, GCOLS], F32, tag="accA", bufs=1)
            accB = psum.tile([128, GCOLS], F32, tag="accB", bufs=1)
            for tx in range(3):
                base = P_OUT0 + gs - WP + (tx - 1)
                rhs = dr_window(base, gn)
                nc.tensor.matmul(accA[:, :gn], w8conv[:, tx, :, 0:128], rhs,
                                 start=(tx == 0), stop=(tx == 2),
                                 perf_mode=DR)
                nc.tensor.matmul(accB[:, :gn], w8conv[:, tx, :, 128:256], rhs,
                                 start=(tx == 0), stop=(tx == 2),
                                 perf_mode=DR)

            # ---- offsets + biases -> SBUF bf16 (one op, bias on ACT)
            # row layout: 0:52 = x-half, 64:116 = y-half (32-aligned bases)
            oxy = work.tile([116, GCOLS], BF16, tag="oxy")
            nc.scalar.activation(oxy[:, :gn], accA[0:116, :gn], AF.Identity,
                                 scale=1.0 / W8SCALE, bias=bA)
            # ---- e = exp(u+bu)*sigmoid((u+bu)/tau), via tanh (same ACT
            #      table as exp): e = (exp(u+bu)/2) * (1 + tanh(5(u+bu)))
            th = work.tile([116, GCOLS], BF16, tag="th")
            nc.scalar.activation(th[:, :gn], accB[0:116, :gn], AF.Tanh,
                                 scale=5.0 / W8SCALE, bias=bT)
            e2 = work.tile([116, GCOLS], BF16, tag="e2")
            nc.scalar.activation(e2[:, :gn], accB[0:116, :gn], AF.Exp,
                                 scale=1.0 / W8SCALE, bias=bE)
            ev = work.tile([116, GCOLS], BF16, tag="ev")
            nc.vector.scalar_tensor_tensor(ev[:, :gn], th[:, :gn], 1.0,
                                           e2[:, :gn], OP.add, OP.mult)

            # ---- monomials (signs folded into g5)
            # rp = [relu(ox); relu(oy)], rn = [min(ox,0); min(oy,0)]
            rp = work.tile([116, GCOLS], BF16, tag="rp")
            nc.vector.tensor_scalar_max(rp[:, :gn], oxy[:, :gn], 0.0)
            rn = work.tile([116, GCOLS], BF16, tag="rn")
            nc.vector.tensor_scalar_min(rn[:, :gn], oxy[:, :gn], 0.0)
            # A2 = [M01; M10], B2 = [M02; M20]
            A2 = work.tile([116, GCOLS], BF16, tag="A2")
            nc.vector.tensor_mul(A2[:, :gn], rp[:, :gn], ev[:, :gn])
            B2 = work.tile([116, GCOLS], BF16, tag="B2")
            nc.vector.tensor_mul(B2[:, :gn], rn[:, :gn], ev[:, :gn])
            # x-half relu factors shifted to base 64 so the second-order
            # products keep equal input base partitions (BIR constraint)
            rx = work.tile([116, GCOLS], BF16, tag="rx")
            nc.vector.tensor_copy(rx[64:116, :gn], rp[0:52, :gn])
            rxn = work.tile([116, GCOLS], BF16, tag="rxn")
            nc.vector.tensor_copy(rxn[64:116, :gn], rn[0:52, :gn])
            # second-order products, all at base 64
            C11 = work.tile([116, GCOLS], BF16, tag="C11")   # M11 @64
            nc.vector.tensor_mul(C11[64:116, :gn], rx[64:116, :gn],
                                 A2[64:116, :gn])
            C21 = work.tile([116, GCOLS], BF16, tag="C21")   # M21 @64
            nc.vector.tensor_mul(C21[64:116, :gn], rx[64:116, :gn],
                                 B2[64:116, :gn])
            D12 = work.tile([116, GCOLS], BF16, tag="D12")   # M12 @64
            nc.vector.tensor_mul(D12[64:116, :gn], rxn[64:116, :gn],
                                 A2[64:116, :gn])
            D22 = work.tile([116, GCOLS], BF16, tag="D22")   # M22 @64
            nc.vector.tensor_mul(D22[64:116, :gn], rxn[64:116, :gn],
                                 B2[64:116, :gn])

            # ---- 7 G-matmuls -> a2 [26, gn]
            a2 = psumA.tile([NTAPD, GCOLS], F32, tag="aT", bufs=1)
            gmm = [(ev[0:52, :gn], g5[0:52, 0, :]),
                   (A2[:, :gn], g5[:, 1, :]),
                   (B2[:, :gn], g5[:, 2, :]),
                   (C11[64:116, :gn], g5[64:116, 3, :]),
                   (C21[64:116, :gn], g5[64:116, 4, :]),
                   (D12[64:116, :gn], g5[64:116, 5, :]),
                   (D22[64:116, :gn], g5[64:116, 6, :])]
            for k, (mono, gm) in enumerate(gmm):
                nc.tensor.matmul(a2[:, :gn], gm, mono,
                                 start=(k == 0), stop=(k == 6))
            a_cm = work.tile([NTAPD, GCOLS], BF16, tag="a_cm")
            nc.scalar.activation(a_cm[:, :gn], a2[:, :gn], AF.Copy)

            # ---- pixel-major A via PE transposes
            a_pm_ps = psumA.tile([128, SCAT_BLKS * NTAPP], BF16, tag="aT",
                                 bufs=1)
            for bo in range(SCAT_BLKS):
                nc.tensor.transpose(
                    a_pm_ps[:, bo * NTAPP : bo * NTAPP + NTAPD],
                    a_cm[:, bo * 128 : (bo + 1) * 128],
                    ident[0:NTAPD, 0:NTAPD])
            a_pm = work.tile([128, SCAT_BLKS, NTAPP], BF16, tag="a_pm")
            nc.vector.tensor_copy(a_pm, a_pm_ps)

            # ---- normalize by denominator
            den = small.tile([128, SCAT_BLKS], F32, tag="den")
            nc.vector.tensor_copy(den, a_pm[:, :, 25])
            if gn < GCOLS:
                nc.vector.memset(den[64:, nblk_g - 1 :], 1.0)
            recip = small.tile([128, SCAT_BLKS], F32, tag="recip")
            nc.vector.reciprocal(recip, den)
            for bo in range(SCAT_BLKS):
                nc.vector.tensor_scalar_mul(a_pm[:, bo, 0:NTAP],
                                            a_pm[:, bo, 0:NTAP],
                                            recip[:, bo : bo + 1])

            # ---- scatter -> S^T
            st = work.tile([128, SCAT_BLKS * QSPAN], BF16, tag="st")
            nc.gpsimd.local_scatter(st, a_pm, sidx[:, grp, :], channels=128,
                                    num_elems=SCAT_BLKS * QSPAN,
                                    num_idxs=SCAT_BLKS * NTAPP)

            # ---- gather (one merged agg tile per group)
            aggf = psumT.tile([OUTC, GCOLS], F32, tag="aggf", bufs=1)
            for bo in range(nblk_g):
                b = grp * SCAT_BLKS + bo
                s_ps = psumT.tile([128, 512], BF16, tag="gat", bufs=2)
                for qc in range(4):
                    nc.tensor.transpose(
                        s_ps[:, qc * 128 : (qc + 1) * 128],
                        st[:, bo * QSPAN + qc * 128 : bo * QSPAN + (qc + 1) * 128],
                        ident)
                schunk = schunkp.tile([128, 512], F8, tag="schunk")
                if bo % 2 == 0:
                    nc.scalar.activation(schunk, s_ps, AF.Copy)
                else:
                    nc.vector.tensor_copy(schunk, s_ps)
                for i in range(2):
                    nc.tensor.matmul(
                        aggf[:, bo * 128 : (bo + 1) * 128],
                        y0_pm[:, b + 1 + 2 * i : b + 3 + 2 * i, :],
                        schunk[:, i * 256 : (i + 1) * 256].rearrange(
                            "p (a b) -> p a b", a=2),
                        start=(i == 0), stop=(i == 1), perf_mode=DR)
            out_cm = work.tile([OUTC, GCOLS], BF16, tag="out_cm")
            nc.scalar.activation(out_cm[:, : nblk_g * 128],
                                 aggf[:, : nblk_g * 128], AF.Copy)

            # ---- MLP + residual (biases via ACT / folded into xres)
            acc1 = psum.tile([OUTC, GCOLS], F32, tag="ps_mm", bufs=2)
            nc.tensor.matmul(acc1[:, :gn], w1T, out_cm[:, :gn], start=True,
                             stop=True)
            h1 = work.tile([OUTC, GCOLS], BF16, tag="h1")
            nc.scalar.activation(h1[:, :gn], acc1[:, :gn], AF.Relu, bias=b1)
            acc2 = psum.tile([OUTC, GCOLS], F32, tag="ps_mm", bufs=2)
            nc.tensor.matmul(acc2[:, :gn], w2T, h1[:, :gn], start=True,
                             stop=True)
            outt = work.tile([OUTC, GCOLS], F32, tag="outt")
            nc.vector.tensor_add(outt[:, :gn], acc2[:, :gn], xres[:, gs:ge])
            nc.sync.dma_start(out=d["out"][:, gs:ge], in_=outt[:, :gn])


# =====================================================================
# Sync-wait legalizer (walrus CoreV3: max 1 SyncWait per instruction)
# =====================================================================

def _legalize_sync_waits(nc, maxw=1):
    f = nc.m.functions[0]
    inserted = 0
    for bb in list(f.blocks):
        out = []
        changed = False
        for inst in bb.instructions:
            si = inst.sync_info
            if si is not None and si.on_wait and len(si.on_wait) > maxw:
                waits = list(si.on_wait)
                best, order = {}, []
                for w in waits:
                    if w.id not in best:
                        best[w.id] = w
                        order.append(w.id)
                    elif w.wait_value > best[w.id].wait_value:
                        best[w.id] = w
                waits = [best[k] for k in order]
                keep, rest = waits[:maxw], waits[maxw:]
                for w in rest:
                    n = mybir.InstNoOp(name=f"I-lg{nc.next_id()}", ins=[], outs=[])
                    n.engine = inst.engine
                    n.sync_info = mybir.SyncInfo(on_wait=[w], on_update=[])
                    out.append(n)
                    inserted += 1
                si.on_wait = keep
                changed = True
            out.append(inst)
        if changed:
            bb.instructions = out
    return inserted


# =====================================================================
# Host-side preparation
# =====================================================================

def _bf(x):
    return np.ascontiguousarray(np.asarray(x, np.float32).astype(ml_dtypes.bfloat16))


def _f32(x):
    return np.ascontiguousarray(np.asarray(x, np.float32))


def _pad_img(img):
    """(C,H,W) f32 -> (C, H+8, WP) with 4 zero rows top/bottom, 1 col each side."""
    c, h, w = img.shape
    out = np.zeros((c, h + 8, WP), np.float32)
    out[:, 4 : 4 + h, 1 : 1 + w] = img
    return out


def _build_slab(xp, r0):
    """X2 [128, P_SLAB] f32: top = rows [r0-2, r0+38), bottom = top + 2 rows."""
    top = xp[:, r0 + 2 : r0 + 42, :].reshape(C, -1)
    bot = xp[:, r0 + 4 : r0 + 44, :].reshape(C, -1)
    x2 = np.zeros((128, P_SLAB), np.float32)
    x2[0:64, LEAD : LEAD + top.shape[1]] = top
    x2[64:128, LEAD : LEAD + bot.shape[1]] = bot
    return x2


def _tap_deltas():
    return [ty * WP + tx for ty in range(-1, 4) for tx in range(-1, 4)]


def _prep_static(p_n, dwf_w, dwf_b, pwf_w, pwf_b, dwc_w, dwc_b, pwc_w, pwc_b,
                 dwm_w, dwm_b, pwm_w, pwm_b, pc_w, pc_b,
                 mlp_w1, mlp_b1, mlp_w2, mlp_b2):
    p_n = np.asarray(p_n, np.float32)
    px = p_n[0].astype(np.int64)
    py = p_n[1].astype(np.int64)
    assert px.min() >= 0 and px.max() <= 2 and py.min() >= 0 and py.max() <= 2

    # ---- composite conv weights W[tap(3x3), c, m] ----
    P_off = np.concatenate([pwf_w[:, :, 0, 0], pwc_w[:, :, 0, 0]], 0)  # [104, 64]
    nf = pwf_w.shape[0]
    dw_off = np.zeros((104, C, 3, 3), np.float32)
    dw_off[0:nf] = dwf_w[:, 0][None, :, :, :]
    dw_off[nf:104] = dwc_w[:, 0][None, :, :, :]
    db_off = np.zeros((104, C), np.float32)
    db_off[0:nf] = dwf_b[None, :]
    db_off[nf:104] = dwc_b[None, :]

    pwm2 = pwm_w[:, :, 0, 0]
    P_u = pwm2[0:NSAMP] - pwm2[NSAMP : NSAMP + 1]
    b_u0 = pwm_b[0:NSAMP] - pwm_b[NSAMP]

    # Wc[t, c, m]: m 0:52 = ox, 52:104 = oy, 104:156 = u
    Wc = np.zeros((9, C, 156), np.float32)
    for t in range(9):
        dy, dx = t // 3 - 1, t % 3 - 1
        Wc[t, :, 0:104] = (P_off * dw_off[:, :, dy + 1, dx + 1]).T
        Wc[t, :, 104:156] = (P_u * dwm_w[:, 0, dy + 1, dx + 1][None, :]).T
    bA = np.concatenate([pwf_b, pwc_b]) + (P_off * db_off).sum(1)   # [104]
    bu = b_u0 + (P_u * dwm_b[None, :]).sum(1)                       # [52]

    # A-block: ox at rows 0:52, oy at rows 64:116 (32-aligned bases);
    # B-block: u duplicated at rows 0:52 and 64:116.
    WcA = np.zeros((9, C, 128), np.float32)
    WcA[:, :, 0:52] = Wc[:, :, 0:52]
    WcA[:, :, 64:116] = Wc[:, :, 52:104]
    WcB = np.zeros((9, C, 128), np.float32)
    WcB[:, :, 0:52] = Wc[:, :, 104:156]
    WcB[:, :, 64:116] = Wc[:, :, 104:156]
    WcAB = np.concatenate([WcA, WcB], axis=2)           # [9, C, 256]
    wconv = np.zeros((128, 6, 256), np.float32)
    for g in range(3):
        tx = g - 1
        wconv[0:64, g, :] = WcAB[0 * 3 + tx + 1]
        wconv[64:128, g, :] = WcAB[2 * 3 + tx + 1]
    for g in range(3, 6):
        tx = g - 4
        wconv[0:64, g, :] = WcAB[1 * 3 + tx + 1]

    # ---- G matrices over monomials ----
    fac = {
        0: {2: -1.0},
        1: {0: 1.0, 1: -1.0, 2: 1.0},
        2: {1: 1.0},
    }
    G = np.zeros((NSAMP, 9, NTAPD), np.float32)
    for n in range(NSAMP):
        for i in range(3):
            for j in range(3):
                ty = py[n] + (i - 1)
                tx = px[n] + (j - 1)
                tap = (ty + 1) * 5 + (tx + 1)
                for a, ca in fac[i].items():
                    for b, cb in fac[j].items():
                        G[n, 3 * a + b, tap] += ca * cb
    G[:, 0, 25] = 1.0

    # stacked G-matrices (logical view for the sim): k=0 ev (rows 0:52);
    # k=1 [M01@0; M10@64]; k=2 [M02@0; M20@64]; k=3..6 single monomials
    # at rows 64:116 (M11, M21, M12, M22)
    g5 = np.zeros((116, 7, NTAPD), np.float32)
    g5[0:52, 0] = G[:, 0]
    g5[0:52, 1] = G[:, 1]
    g5[64:116, 1] = G[:, 3]
    g5[0:52, 2] = G[:, 2]
    g5[64:116, 2] = G[:, 6]
    g5[64:116, 3] = G[:, 4]
    g5[64:116, 4] = G[:, 7]
    g5[64:116, 5] = G[:, 5]
    g5[64:116, 6] = G[:, 8]


    # ---- scatter indices ----
    deltas = _tap_deltas()
    sidx = np.zeros((128, NSCAT, SCAT_BLKS * NTAPP), np.int16)
    for p in range(128):
        negctr = 1
        for sct in range(NSCAT):
            for boff in range(SCAT_BLKS):
                b = sct * SCAT_BLKS + boff
                for j in range(NTAPP):
                    col = boff * NTAPP + j
                    if b >= NBLK or j >= NTAP:
                        sidx[p, sct, col] = -negctr
                        negctr += 1
                    else:
                        sidx[p, sct, col] = boff * QSPAN + p + deltas[j] + 67
    assert sidx.max() < SCAT_BLKS * QSPAN

    # ---- small weights / blobs ----
    pcT = pc_w[:, :, 0, 0].T
    w1T = mlp_w1.T
    w2T = mlp_w2.T
    b1p = mlp_b1 + mlp_w1 @ pc_b
    b2p = mlp_b2

    wb = np.zeros((128, WB_COLS), np.float32)
    wb[:, WB_IDENT : WB_IDENT + 128] = np.eye(128, dtype=np.float32)
    wb[0:52, WB_GEV : WB_GEV + 26] = G[:, 0]
    wb[0:C, WB_PCT : WB_PCT + 64] = pcT
    wb[0:OUTC, WB_W1T : WB_W1T + 64] = w1T
    wb[0:OUTC, WB_W2T : WB_W2T + 64] = w2T
    wb[0:116, WB_G5 : WB_G5 + 182] = g5.reshape(116, -1)

    # fp8 blob: conv DR pairs (scaled)
    wc8 = np.zeros((128, 3, 2, 256), np.float32)
    for tx in range(3):
        wc8[:, tx, 0, :] = wconv[:, tx, :] * W8SCALE        # rows r-1/r+1
        wc8[:, tx, 1, :] = wconv[:, 3 + tx, :] * W8SCALE    # center row
    wb8 = np.zeros((128, W8_COLS), np.float32)
    wb8[:, W8_CONV:W8_COLS] = wc8.reshape(128, -1)

    wbf32 = np.zeros((128, 4), np.float32)
    wbf32[0:52, 0] = bA[0:52]
    wbf32[64:116, 0] = bA[52:104]
    for col, vec in ((1, 5.0 * bu), (2, bu - np.log(2.0))):
        wbf32[0:52, col] = vec
        wbf32[64:116, col] = vec
    wbf32[0:OUTC, 3] = b1p

    def _f8(a):
        return np.ascontiguousarray(
            np.asarray(a, np.float32).astype(ml_dtypes.float8_e4m3fn))

    return {
        "wb16": _bf(wb),
        "wb8": _f8(wb8),
        "wbf32": _f32(wbf32),
        "sidx": sidx,
        # logical views for the numpy sim:
        "wconv": wconv,
        "bA": _f32(bA),
        "bu": _f32(bu),
        "g5": g5,
        "G": G,
        "pcT": pcT,
        "w1T": w1T,
        "w2T": w2T,
        "b1": _f32(b1p).reshape(OUTC, 1),
        "b2": _f32(b2p).reshape(OUTC, 1),
    }


def _build_nc():
    nc = bass.Bass()
    d = {}
    d["x2"] = nc.dram_tensor("x2", [128, P_SLAB], BF16, kind="ExternalInput")
    d["x8"] = nc.dram_tensor("x8", [128, P_SLAB], F8, kind="ExternalInput")
    d["xres"] = nc.dram_tensor("xres", [C, NP_OUT], F32, kind="ExternalInput")
    d["wb16"] = nc.dram_tensor("wb16", [128, WB_COLS], BF16, kind="ExternalInput")
    d["wb8"] = nc.dram_tensor("wb8", [128, W8_COLS], F8, kind="ExternalInput")
    d["wbf32"] = nc.dram_tensor("wbf32", [128, 4], F32, kind="ExternalInput")
    d["sidx"] = nc.dram_tensor("sidx", [128, NSCAT, SCAT_BLKS * NTAPP], I16,
                               kind="ExternalInput")
    d["out"] = nc.dram_tensor("out", [C, NP_OUT], F32, kind="ExternalOutput")

    with tile.TileContext(nc) as tc:
        _emit(nc, tc, d)

    lower_extended_insts(nc)
    _legalize_sync_waits(nc)
    return nc


def _get_nc():
    if "nc" not in _CACHE:
        _CACHE["nc"] = _build_nc()
    return _CACHE["nc"]


def kernel(x, p_n, dwf_w, dwf_b, pwf_w, pwf_b, dwc_w, dwc_b, pwc_w, pwc_b,
           dwm_w, dwm_b, pwm_w, pwm_b, pc_w, pc_b, mlp_w1, mlp_b1, mlp_w2,
           mlp_b2, _bench=None):
    x = np.asarray(x, np.float32)
    stat = _prep_static(
        np.asarray(p_n), np.asarray(dwf_w, np.float32),
        np.asarray(dwf_b, np.float32), np.asarray(pwf_w, np.float32),
        np.asarray(pwf_b, np.float32), np.asarray(dwc_w, np.float32),
        np.asarray(dwc_b, np.float32), np.asarray(pwc_w, np.float32),
        np.asarray(pwc_b, np.float32), np.asarray(dwm_w, np.float32),
        np.asarray(dwm_b, np.float32), np.asarray(pwm_w, np.float32),
        np.asarray(pwm_b, np.float32), np.asarray(pc_w, np.float32),
        np.asarray(pc_b, np.float32), np.asarray(mlp_w1, np.float32),
        np.asarray(mlp_b1, np.float32), np.asarray(mlp_w2, np.float32),
        np.asarray(mlp_b2, np.float32),
    )

    in_maps = []
    shards = []
    b2 = stat["b2"][:, 0]
    for core in range(N_CORES):
        bidx, half = divmod(core, 2)
        r0 = half * ROWS_OUT
        shards.append((bidx, r0))
        xp = _pad_img(x[bidx])
        x2 = _build_slab(xp, r0)
        xres = np.zeros((C, NP_OUT), np.float32)
        xres += b2[:, None]
        xres.reshape(C, ROWS_OUT, WP)[:, :, 1 : 1 + W] += \
            x[bidx, :, r0 : r0 + ROWS_OUT, :]
        m = {"wb16": stat["wb16"], "wb8": stat["wb8"],
             "wbf32": stat["wbf32"], "sidx": stat["sidx"],
             "x2": _bf(x2),
             "x8": np.ascontiguousarray(
                 np.asarray(x2, np.float32).astype(ml_dtypes.float8_e4m3fn)),
             "xres": _f32(xres)}
        in_maps.append(m)

    nc = _get_nc()
    kw = dict(_bench) if _bench else {}
    res = run_bass_kernel_spmd(nc, in_maps, list(range(N_CORES)), **kw)

    out = np.zeros((B, OUTC, H, W), np.float32)
    for core, (bidx, r0) in enumerate(shards):
        o = res.results[core]["out"].reshape(OUTC, ROWS_OUT, WP)
        out[bidx, :, r0 : r0 + ROWS_OUT, :] = o[:, :, 1 : 1 + W]
    if _bench is not None:
        _CACHE["last_results"] = res
    return out


# revision 26
# speedup vs baseline: 1.1041x; 1.1041x over previous
"""Trainium2 Bass kernel for nn_CrossDConv (sparse deformable attention conv).

Self-contained: host-side sharding/layout prep + Bass/Tile kernel, SPMD on
8 NeuronCores via run_bass_kernel_spmd.  Each core handles one
(batch, row-half) shard of the (4, 64, 64, 64) input.

All device work runs in a width-padded pixel space (66-wide rows, one zero
column each side, plus zero rows above/below the shard) so 3x3-conv taps
and bilinear-gather taps never wrap across rows: zero padding reproduces
the reference's conv zero-padding and zero-padded bilinear sampling
exactly, with no masks.

Math restructuring (exact, host-side):
  * Both depthwise3x3+pointwise1x1 offset branches and the modulation
    branch fuse into ONE composite 3x3 conv; outputs are packed as
    A = [ox; oy] (104 rows) and B = [u; u] (u duplicated, 104 rows) with
    u = scores - sparsity (softmax shift-invariance).  Biases are applied
    by the scalar engine (activation bias operand), not matmuls.
  * Unnormalized softmax weight e = exp(u)*sigmoid(u/tau), computed as
    e = (exp(u+bu)/2) * (1 + tanh(5*(u+bu))): tanh and exp share one
    activation table set, so the ACT engine never reloads tables.
  * Bilinear tent weights expanded over monomials {1, relu(t), -relu(-t)};
    the duplicated-row packing lets pairs of monomials stack into
    104-partition tiles, so the 3x3 recombination takes 5 G-matmuls
    (K=52/104) instead of 9; signs fold into static G matrices.
  * The 26th output row of the G-matmuls is the softmax denominator.
  * 1x1 "pc" conv commutes with the gather: the gather runs on
    y0 = pc_w @ x (computed directly pixel-major); pc bias folds into the
    first MLP bias, mlp bias 2 folds into the residual input.
  * Gather as banded matmul: normalized pixel-major A scattered into S^T
    (GPSIMD local_scatter, static indices), PE-transposed into q-major S
    chunks, PE matmuls against pixel-major y0.

The pipeline runs as 6 independent 384-pixel groups so Tile can overlap
phases across groups; all transposes use the PE (DMA-transpose costs
~1.2us of serial Sync-engine dispatch per call on this target).
"""

import numpy as np
import ml_dtypes

import concourse.bass as bass
import concourse.tile as tile
from concourse import mybir, library_config
from concourse.bass_utils import run_bass_kernel_spmd
from concourse.library_overlay import lower_extended_insts

BF16 = mybir.dt.bfloat16
F32 = mybir.dt.float32
F8 = mybir.dt.float8e4
I16 = mybir.dt.int16
W8SCALE = 256.0

# ------------------------------------------------------------------ geometry
B, C, H, W = 4, 64, 64, 64
OUTC = 64
N_CORES = 8
TAU = 0.1
NSAMP = 52
WP = W + 2                      # padded row width
ROWS_OUT = H // 2               # 32 output rows per core
LEAD = 63                       # leading zeros so P_OUT0 = 195 (=67+128)
SLAB_ROWS = 40                  # rows r0-2 .. r0+38 (zero-padded outside image)
P_SLAB = 2816                   # 63 + 40*66 + tail zeros, 22 chunks of 128
P_OUT0 = LEAD + 2 * WP          # 195
NP_OUT = ROWS_OUT * WP          # 2112 padded positions carrying outputs
NBLK = (NP_OUT + 127) // 128    # 17 pixel blocks
QSPAN = 512                     # q-window per block: [p0-67, p0+445)
NTAP = 25
NTAPD = 26
NTAPP = 32                      # padded tap stride
SCAT_BLKS = 3
NSCAT = (NBLK + SCAT_BLKS - 1) // SCAT_BLKS   # 6 groups
GCOLS = SCAT_BLKS * 128         # 384 pixels per group

# bf16 weight blob column layout
WB_IDENT = 0                    # [128, 128]
WB_GEV = 128                    # [52, 26]
WB_PCT = 154                    # [64, 64]
WB_W1T = 218
WB_W2T = 282
WB_G5 = 346                     # [116, 7*26]
WB_COLS = 528
# fp8 weight blob column layout
W8_CONV = 0                     # [128, 3*2*256] DR conv pairs (x W8SCALE)
W8_COLS = 1536

_CACHE = {}


# =====================================================================
# Device kernel
# =====================================================================

def _emit(nc, tc, d):
    from contextlib import ExitStack
    AF = mybir.ActivationFunctionType
    OP = mybir.AluOpType

    with ExitStack() as ctx:
        weights = ctx.enter_context(tc.tile_pool(name="weights", bufs=1))
        big = ctx.enter_context(tc.tile_pool(name="big", bufs=1))
        work = ctx.enter_context(tc.tile_pool(name="work", bufs=3))
        small = ctx.enter_context(tc.tile_pool(name="small", bufs=2))
        schunkp = ctx.enter_context(tc.tile_pool(name="schunk", bufs=4))
        psum = ctx.enter_context(tc.tile_pool(name="psum", bufs=1, space="PSUM"))
        psumA = ctx.enter_context(tc.tile_pool(name="psumA", bufs=2, space="PSUM"))
        psumT = ctx.enter_context(tc.tile_pool(name="psumT", bufs=1, space="PSUM"))

        nc.gpsimd.load_library(library_config.local_scatter)

        # ---------------- merged loads (conv inputs first)
        x8 = big.tile([128, P_SLAB], F8)
        nc.sync.dma_start(out=x8, in_=d["x8"][:, :])
        w8 = weights.tile([128, W8_COLS], F8)
        nc.sync.dma_start(out=w8, in_=d["wb8"][:, :])
        wb = weights.tile([128, WB_COLS], BF16)
        nc.sync.dma_start(out=wb, in_=d["wb16"][:, :])
        wf = weights.tile([128, 4], F32)
        nc.sync.dma_start(out=wf, in_=d["wbf32"][:, :])
        x2 = big.tile([128, P_SLAB], BF16)
        nc.sync.dma_start(out=x2, in_=d["x2"][:, :])
        sidx = weights.tile([128, NSCAT, SCAT_BLKS * NTAPP], I16)
        nc.sync.dma_start(out=sidx, in_=d["sidx"][:, :, :])
        xres = big.tile([C, NP_OUT], F32)
        nc.sync.dma_start(out=xres, in_=d["xres"][:, :])

        ident = wb[:, WB_IDENT : WB_IDENT + 128]
        gev = wb[0:52, WB_GEV : WB_GEV + 26]
        pcT = wb[0:C, WB_PCT : WB_PCT + 64]
        w1T = wb[0:OUTC, WB_W1T : WB_W1T + 64]
        w2T = wb[0:OUTC, WB_W2T : WB_W2T + 64]
        w8conv = w8[:, W8_CONV:W8_COLS].rearrange("p (t a m) -> p t a m",
                                                  t=3, a=2)
        g5 = wb[0:116, WB_G5 : WB_G5 + 182].rearrange("p (k t) -> p k t", k=7)
        bA = wf[0:116, 0:1]
        bT = wf[0:116, 1:2]
        bE = wf[0:116, 2:3]
        b1 = wf[0:OUTC, 3:4]

        def dr_window(base, gn):
            # [128, 2, gn] view of x8 with the two K-tiles WP apart
            sl = x8[:, base : base + gn]
            return bass.AP(tensor=sl.tensor, offset=sl.offset,
                           ap=[list(sl.ap[0]), [WP, 2], [1, gn]])

        # ---------------- y0 pixel-major (fp8), chunks 1..20 (used by gather)
        NQCH = P_SLAB // 128
        y0_pm = big.tile([128, NQCH, OUTC], F8)

        def emit_y0():
            for qg in range(5):
                accy = psum.tile([128, 4 * OUTC], F32, tag="ps_mm", bufs=2)
                for qi in range(4):
                    qc = 1 + qg * 4 + qi
                    s = qc * 128
                    nc.tensor.matmul(accy[:, qi * OUTC : (qi + 1) * OUTC],
                                     x2[0:C, s : s + 128], pcT, start=True,
                                     stop=True)
                if qg % 2 == 0:
                    nc.scalar.activation(y0_pm[:, 1 + qg * 4 : 5 + qg * 4, :],
                                         accy, AF.Copy)
                else:
                    nc.vector.tensor_copy(
                        y0_pm[:, 1 + qg * 4 : 5 + qg * 4, :], accy)

        # ---------------- main per-group pipeline
        DR = mybir.MatmulPerfMode.DoubleRow
        for grp in range(NSCAT):
            gs = grp * GCOLS
            ge = min(gs + GCOLS, NP_OUT)
            gn = ge - gs
            nblk_g = min(SCAT_BLKS, NBLK - grp * SCAT_BLKS)

            # ---- composite conv, fp8 DoubleRow: 3 matmuls per branch,
            #      K-tiles = (rows r-1/r+1, stacked) and (center row, WP away)
            accA = psum.tile([128, GCOLS], F32, tag="accA", bufs=1)
            accB = psum.tile([128, GCOLS], F32, tag="accB", bufs=1)
            for tx in range(3):
                base = P_OUT0 + gs - WP + (tx - 1)
                rhs = dr_window(base, gn)
                nc.tensor.matmul(accA[:, :gn], w8conv[:, tx, :, 0:128], rhs,
                                 start=(tx == 0), stop=(tx == 2),
                                 perf_mode=DR)
                nc.tensor.matmul(accB[:, :gn], w8conv[:, tx, :, 128:256], rhs,
                                 start=(tx == 0), stop=(tx == 2),
                                 perf_mode=DR)

            # ---- offsets + biases -> SBUF bf16 (one op, bias on ACT)
            # row layout: 0:52 = x-half, 64:116 = y-half (32-aligned bases)
            oxy = work.tile([116, GCOLS], BF16, tag="oxy")
            nc.scalar.activation(oxy[:, :gn], accA[0:116, :gn], AF.Identity,
                                 scale=1.0 / W8SCALE, bias=bA)
            # ---- e = exp(u+bu)*sigmoid((u+bu)/tau), via tanh (same ACT
            #      table as exp): e = (exp(u+bu)/2) * (1 + tanh(5(u+bu)))
            th = work.tile([116, GCOLS], BF16, tag="th")
            nc.scalar.activation(th[:, :gn], accB[0:116, :gn], AF.Tanh,
                                 scale=5.0 / W8SCALE, bias=bT)
            e2 = work.tile([116, GCOLS], BF16, tag="e2")
            nc.scalar.activation(e2[:, :gn], accB[0:116, :gn], AF.Exp,
                                 scale=1.0 / W8SCALE, bias=bE)
            ev = work.tile([116, GCOLS], BF16, tag="ev")
            nc.vector.scalar_tensor_tensor(ev[:, :gn], th[:, :gn], 1.0,
                                           e2[:, :gn], OP.add, OP.mult)

            # ---- monomials (signs folded into g5)
            # rp = [relu(ox); relu(oy)], rn = [min(ox,0); min(oy,0)]
            rp = work.tile([116, GCOLS], BF16, tag="rp")
            nc.vector.tensor_scalar_max(rp[:, :gn], oxy[:, :gn], 0.0)
            rn = work.tile([116, GCOLS], BF16, tag="rn")
            nc.vector.tensor_scalar_min(rn[:, :gn], oxy[:, :gn], 0.0)
            # A2 = [M01; M10], B2 = [M02; M20]
            A2 = work.tile([116, GCOLS], BF16, tag="A2")
            nc.vector.tensor_mul(A2[:, :gn], rp[:, :gn], ev[:, :gn])
            B2 = work.tile([116, GCOLS], BF16, tag="B2")
            nc.vector.tensor_mul(B2[:, :gn], rn[:, :gn], ev[:, :gn])
            # x-half relu factors shifted to base 64 so the second-order
            # products keep equal input base partitions (BIR constraint)
            rx = work.tile([116, GCOLS], BF16, tag="rx")
            nc.vector.tensor_copy(rx[64:116, :gn], rp[0:52, :gn])
            rxn = work.tile([116, GCOLS], BF16, tag="rxn")
            nc.vector.tensor_copy(rxn[64:116, :gn], rn[0:52, :gn])
            # second-order products, all at base 64
            C11 = work.tile([116, GCOLS], BF16, tag="C11")   # M11 @64
            nc.vector.tensor_mul(C11[64:116, :gn], rx[64:116, :gn],
                                 A2[64:116, :gn])
            C21 = work.tile([116, GCOLS], BF16, tag="C21")   # M21 @64
            nc.vector.tensor_mul(C21[64:116, :gn], rx[64:116, :gn],
                                 B2[64:116, :gn])
            D12 = work.tile([116, GCOLS], BF16, tag="D12")   # M12 @64
            nc.vector.tensor_mul(D12[64:116, :gn], rxn[64:116, :gn],
                                 A2[64:116, :gn])
            D22 = work.tile([116, GCOLS], BF16, tag="D22")   # M22 @64
            nc.vector.tensor_mul(D22[64:116, :gn], rxn[64:116, :gn],
                                 B2[64:116, :gn])

            # ---- 7 G-matmuls -> a2 [26, gn]
            a2 = psumA.tile([NTAPD, GCOLS], F32, tag="aT", bufs=1)
            gmm = [(ev[0:52, :gn], g5[0:52, 0, :]),
                   (A2[:, :gn], g5[:, 1, :]),
                   (B2[:, :gn], g5[:, 2, :]),
                   (C11[64:116, :gn], g5[64:116, 3, :]),
                   (C21[64:116, :gn], g5[64:116, 4, :]),
                   (D12[64:116, :gn], g5[64:116, 5, :]),
                   (D22[64:116, :gn], g5[64:116, 6, :])]
            for k, (mono, gm) in enumerate(gmm):
                nc.tensor.matmul(a2[:, :gn], gm, mono,
                                 start=(k == 0), stop=(k == 6))
            a_cm = work.tile([NTAPD, GCOLS], BF16, tag="a_cm")
            nc.scalar.activation(a_cm[:, :gn], a2[:, :gn], AF.Copy)

            # ---- pixel-major A via PE transposes
            a_pm_ps = psumA.tile([128, SCAT_BLKS * NTAPP], BF16, tag="aT",
                                 bufs=1)
            for bo in range(SCAT_BLKS):
                nc.tensor.transpose(
                    a_pm_ps[:, bo * NTAPP : bo * NTAPP + NTAPD],
                    a_cm[:, bo * 128 : (bo + 1) * 128],
                    ident[0:NTAPD, 0:NTAPD])
            a_pm = work.tile([128, SCAT_BLKS, NTAPP], BF16, tag="a_pm")
            nc.vector.tensor_copy(a_pm, a_pm_ps)

            # ---- normalize by denominator
            den = small.tile([128, SCAT_BLKS], F32, tag="den")
            nc.vector.tensor_copy(den, a_pm[:, :, 25])
            if gn < GCOLS:
                nc.vector.memset(den[64:, nblk_g - 1 :], 1.0)
            recip = small.tile([128, SCAT_BLKS], F32, tag="recip")
            nc.vector.reciprocal(recip, den)
            for bo in range(SCAT_BLKS):
                nc.vector.tensor_scalar_mul(a_pm[:, bo, 0:NTAP],
                                            a_pm[:, bo, 0:NTAP],
                                            recip[:, bo : bo + 1])

            # ---- scatter -> S^T
            st = work.tile([128, SCAT_BLKS * QSPAN], BF16, tag="st")
            nc.gpsimd.local_scatter(st, a_pm, sidx[:, grp, :], channels=128,
                                    num_elems=SCAT_BLKS * QSPAN,
                                    num_idxs=SCAT_BLKS * NTAPP)

            if grp == 0:
                emit_y0()

            # ---- gather (one merged agg tile per group)
            aggf = psumT.tile([OUTC, GCOLS], F32, tag="aggf", bufs=1)
            for bo in range(nblk_g):
                b = grp * SCAT_BLKS + bo
                s_ps = psumT.tile([128, 512], BF16, tag="gat", bufs=2)
                for qc in range(4):
                    nc.tensor.transpose(
                        s_ps[:, qc * 128 : (qc + 1) * 128],
                        st[:, bo * QSPAN + qc * 128 : bo * QSPAN + (qc + 1) * 128],
                        ident)
                schunk = schunkp.tile([128, 512], F8, tag="schunk")
                if bo % 2 == 0:
                    nc.scalar.activation(schunk, s_ps, AF.Copy)
                else:
                    nc.vector.tensor_copy(schunk, s_ps)
                for i in range(2):
                    nc.tensor.matmul(
                        aggf[:, bo * 128 : (bo + 1) * 128],
                        y0_pm[:, b + 1 + 2 * i : b + 3 + 2 * i, :],
                        schunk[:, i * 256 : (i + 1) * 256].rearrange(
                            "p (a b) -> p a b", a=2),
                        start=(i == 0), stop=(i == 1), perf_mode=DR)
            out_cm = work.tile([OUTC, GCOLS], BF16, tag="out_cm")
            nc.scalar.activation(out_cm[:, : nblk_g * 128],
                                 aggf[:, : nblk_g * 128], AF.Copy)

            # ---- MLP + residual (biases via ACT / folded into xres)
            acc1 = psum.tile([OUTC, GCOLS], F32, tag="ps_mm", bufs=2)
            nc.tensor.matmul(acc1[:, :gn], w1T, out_cm[:, :gn], start=True,
                             stop=True)
            h1 = work.tile([OUTC, GCOLS], BF16, tag="h1")
            nc.scalar.activation(h1[:, :gn], acc1[:, :gn], AF.Relu, bias=b1)
            acc2 = psum.tile([OUTC, GCOLS], F32, tag="ps_mm", bufs=2)
            nc.tensor.matmul(acc2[:, :gn], w2T, h1[:, :gn], start=True,
                             stop=True)
            outt = work.tile([OUTC, GCOLS], F32, tag="outt")
            nc.vector.tensor_add(outt[:, :gn], acc2[:, :gn], xres[:, gs:ge])
            nc.sync.dma_start(out=d["out"][:, gs:ge], in_=outt[:, :gn])


# =====================================================================
# Sync-wait legalizer (walrus CoreV3: max 1 SyncWait per instruction)
# =====================================================================

def _legalize_sync_waits(nc, maxw=1):
    f = nc.m.functions[0]
    inserted = 0
    for bb in list(f.blocks):
        out = []
        changed = False
        for inst in bb.instructions:
            si = inst.sync_info
            if si is not None and si.on_wait and len(si.on_wait) > maxw:
                waits = list(si.on_wait)
                best, order = {}, []
                for w in waits:
                    if w.id not in best:
                        best[w.id] = w
                        order.append(w.id)
                    elif w.wait_value > best[w.id].wait_value:
                        best[w.id] = w
                waits = [best[k] for k in order]
                keep, rest = waits[:maxw], waits[maxw:]
                for w in rest:
                    n = mybir.InstNoOp(name=f"I-lg{nc.next_id()}", ins=[], outs=[])
                    n.engine = inst.engine
                    n.sync_info = mybir.SyncInfo(on_wait=[w], on_update=[])
                    out.append(n)
                    inserted += 1
                si.on_wait = keep
                changed = True
            out.append(inst)
        if changed:
            bb.instructions = out
    return inserted


# =====================================================================
# Host-side preparation
# =====================================================================

def _bf(x):
    return np.ascontiguousarray(np.asarray(x, np.float32).astype(ml_dtypes.bfloat16))


def _f32(x):
    return np.ascontiguousarray(np.asarray(x, np.float32))


def _pad_img(img):
    """(C,H,W) f32 -> (C, H+8, WP) with 4 zero rows top/bottom, 1 col each side."""
    c, h, w = img.shape
    out = np.zeros((c, h + 8, WP), np.float32)
    out[:, 4 : 4 + h, 1 : 1 + w] = img
    return out


def _build_slab(xp, r0):
    """X2 [128, P_SLAB] f32: top = rows [r0-2, r0+38), bottom = top + 2 rows."""
    top = xp[:, r0 + 2 : r0 + 42, :].reshape(C, -1)
    bot = xp[:, r0 + 4 : r0 + 44, :].reshape(C, -1)
    x2 = np.zeros((128, P_SLAB), np.float32)
    x2[0:64, LEAD : LEAD + top.shape[1]] = top
    x2[64:128, LEAD : LEAD + bot.shape[1]] = bot
    return x2


def _tap_deltas():
    return [ty * WP + tx for ty in range(-1, 4) for tx in range(-1, 4)]


def _prep_static(p_n, dwf_w, dwf_b, pwf_w, pwf_b, dwc_w, dwc_b, pwc_w, pwc_b,
                 dwm_w, dwm_b, pwm_w, pwm_b, pc_w, pc_b,
                 mlp_w1, mlp_b1, mlp_w2, mlp_b2):
    p_n = np.asarray(p_n, np.float32)
    px = p_n[0].astype(np.int64)
    py = p_n[1].astype(np.int64)
    assert px.min() >= 0 and px.max() <= 2 and py.min() >= 0 and py.max() <= 2

    # ---- composite conv weights W[tap(3x3), c, m] ----
    P_off = np.concatenate([pwf_w[:, :, 0, 0], pwc_w[:, :, 0, 0]], 0)  # [104, 64]
    nf = pwf_w.shape[0]
    dw_off = np.zeros((104, C, 3, 3), np.float32)
    dw_off[0:nf] = dwf_w[:, 0][None, :, :, :]
    dw_off[nf:104] = dwc_w[:, 0][None, :, :, :]
    db_off = np.zeros((104, C), np.float32)
    db_off[0:nf] = dwf_b[None, :]
    db_off[nf:104] = dwc_b[None, :]

    pwm2 = pwm_w[:, :, 0, 0]
    P_u = pwm2[0:NSAMP] - pwm2[NSAMP : NSAMP + 1]
    b_u0 = pwm_b[0:NSAMP] - pwm_b[NSAMP]

    # Wc[t, c, m]: m 0:52 = ox, 52:104 = oy, 104:156 = u
    Wc = np.zeros((9, C, 156), np.float32)
    for t in range(9):
        dy, dx = t // 3 - 1, t % 3 - 1
        Wc[t, :, 0:104] = (P_off * dw_off[:, :, dy + 1, dx + 1]).T
        Wc[t, :, 104:156] = (P_u * dwm_w[:, 0, dy + 1, dx + 1][None, :]).T
    bA = np.concatenate([pwf_b, pwc_b]) + (P_off * db_off).sum(1)   # [104]
    bu = b_u0 + (P_u * dwm_b[None, :]).sum(1)                       # [52]

    # A-block: ox at rows 0:52, oy at rows 64:116 (32-aligned bases);
    # B-block: u duplicated at rows 0:52 and 64:116.
    WcA = np.zeros((9, C, 128), np.float32)
    WcA[:, :, 0:52] = Wc[:, :, 0:52]
    WcA[:, :, 64:116] = Wc[:, :, 52:104]
    WcB = np.zeros((9, C, 128), np.float32)
    WcB[:, :, 0:52] = Wc[:, :, 104:156]
    WcB[:, :, 64:116] = Wc[:, :, 104:156]
    WcAB = np.concatenate([WcA, WcB], axis=2)           # [9, C, 256]
    wconv = np.zeros((128, 6, 256), np.float32)
    for g in range(3):
        tx = g - 1
        wconv[0:64, g, :] = WcAB[0 * 3 + tx + 1]
        wconv[64:128, g, :] = WcAB[2 * 3 + tx + 1]
    for g in range(3, 6):
        tx = g - 4
        wconv[0:64, g, :] = WcAB[1 * 3 + tx + 1]

    # ---- G matrices over monomials ----
    fac = {
        0: {2: -1.0},
        1: {0: 1.0, 1: -1.0, 2: 1.0},
        2: {1: 1.0},
    }
    G = np.zeros((NSAMP, 9, NTAPD), np.float32)
    for n in range(NSAMP):
        for i in range(3):
            for j in range(3):
                ty = py[n] + (i - 1)
                tx = px[n] + (j - 1)
                tap = (ty + 1) * 5 + (tx + 1)
                for a, ca in fac[i].items():
                    for b, cb in fac[j].items():
                        G[n, 3 * a + b, tap] += ca * cb
    G[:, 0, 25] = 1.0

    # stacked G-matrices (logical view for the sim): k=0 ev (rows 0:52);
    # k=1 [M01@0; M10@64]; k=2 [M02@0; M20@64]; k=3..6 single monomials
    # at rows 64:116 (M11, M21, M12, M22)
    g5 = np.zeros((116, 7, NTAPD), np.float32)
    g5[0:52, 0] = G[:, 0]
    g5[0:52, 1] = G[:, 1]
    g5[64:116, 1] = G[:, 3]
    g5[0:52, 2] = G[:, 2]
    g5[64:116, 2] = G[:, 6]
    g5[64:116, 3] = G[:, 4]
    g5[64:116, 4] = G[:, 7]
    g5[64:116, 5] = G[:, 5]
    g5[64:116, 6] = G[:, 8]


    # ---- scatter indices ----
    deltas = _tap_deltas()
    sidx = np.zeros((128, NSCAT, SCAT_BLKS * NTAPP), np.int16)
    for p in range(128):
        negctr = 1
        for sct in range(NSCAT):
            for boff in range(SCAT_BLKS):
                b = sct * SCAT_BLKS + boff
                for j in range(NTAPP):
                    col = boff * NTAPP + j
                    if b >= NBLK or j >= NTAP:
                        sidx[p, sct, col] = -negctr
                        negctr += 1
                    else:
                        sidx[p, sct, col] = boff * QSPAN + p + deltas[j] + 67
    assert sidx.max() < SCAT_BLKS * QSPAN

    # ---- small weights / blobs ----
    pcT = pc_w[:, :, 0, 0].T
    w1T = mlp_w1.T
    w2T = mlp_w2.T
    b1p = mlp_b1 + mlp_w1 @ pc_b
    b2p = mlp_b2

    wb = np.zeros((128, WB_COLS), np.float32)
    wb[:, WB_IDENT : WB_IDENT + 128] = np.eye(128, dtype=np.float32)
    wb[0:52, WB_GEV : WB_GEV + 26] = G[:, 0]
    wb[0:C, WB_PCT : WB_PCT + 64] = pcT
    wb[0:OUTC, WB_W1T : WB_W1T + 64] = w1T
    wb[0:OUTC, WB_W2T : WB_W2T + 64] = w2T
    wb[0:116, WB_G5 : WB_G5 + 182] = g5.reshape(116, -1)

    # fp8 blob: conv DR pairs (scaled)
    wc8 = np.zeros((128, 3, 2, 256), np.float32)
    for tx in range(3):
        wc8[:, tx, 0, :] = wconv[:, tx, :] * W8SCALE        # rows r-1/r+1
        wc8[:, tx, 1, :] = wconv[:, 3 + tx, :] * W8SCALE    # center row
    wb8 = np.zeros((128, W8_COLS), np.float32)
    wb8[:, W8_CONV:W8_COLS] = wc8.reshape(128, -1)

    wbf32 = np.zeros((128, 4), np.float32)
    wbf32[0:52, 0] = bA[0:52]
    wbf32[64:116, 0] = bA[52:104]
    for col, vec in ((1, 5.0 * bu), (2, bu - np.log(2.0))):
        wbf32[0:52, col] = vec
        wbf32[64:116, col] = vec
    wbf32[0:OUTC, 3] = b1p

    def _f8(a):
        return np.ascontiguousarray(
            np.asarray(a, np.float32).astype(ml_dtypes.float8_e4m3fn))

    return {
        "wb16": _bf(wb),
        "wb8": _f8(wb8),
        "wbf32": _f32(wbf32),
        "sidx": sidx,
        # logical views for the numpy sim:
        "wconv": wconv,
        "bA": _f32(bA),
        "bu": _f32(bu),
        "g5": g5,
        "G": G,
        "pcT": pcT,
        "w1T": w1T,
        "w2T": w2T,
        "b1": _f32(b1p).reshape(OUTC, 1),
        "b2": _f32(b2p).reshape(OUTC, 1),
    }


def _build_nc():
    nc = bass.Bass()
    d = {}
    d["x2"] = nc.dram_tensor("x2", [128, P_SLAB], BF16, kind="ExternalInput")
    d["x8"] = nc.dram_tensor("x8", [128, P_SLAB], F8, kind="ExternalInput")
    d["xres"] = nc.dram_tensor("xres", [C, NP_OUT], F32, kind="ExternalInput")
    d["wb16"] = nc.dram_tensor("wb16", [128, WB_COLS], BF16, kind="ExternalInput")
    d["wb8"] = nc.dram_tensor("wb8", [128, W8_COLS], F8, kind="ExternalInput")
    d["wbf32"] = nc.dram_tensor("wbf32", [128, 4], F32, kind="ExternalInput")
    d["sidx"] = nc.dram_tensor("sidx", [128, NSCAT, SCAT_BLKS * NTAPP], I16,
                               kind="ExternalInput")
    d["out"] = nc.dram_tensor("out", [C, NP_OUT], F32, kind="ExternalOutput")

    with tile.TileContext(nc) as tc:
        _emit(nc, tc, d)

    lower_extended_insts(nc)
    _legalize_sync_waits(nc)
    return nc


def _get_nc():
    if "nc" not in _CACHE:
        _CACHE["nc"] = _build_nc()
    return _CACHE["nc"]


def kernel(x, p_n, dwf_w, dwf_b, pwf_w, pwf_b, dwc_w, dwc_b, pwc_w, pwc_b,
           dwm_w, dwm_b, pwm_w, pwm_b, pc_w, pc_b, mlp_w1, mlp_b1, mlp_w2,
           mlp_b2, _bench=None):
    x = np.asarray(x, np.float32)
    stat = _prep_static(
        np.asarray(p_n), np.asarray(dwf_w, np.float32),
        np.asarray(dwf_b, np.float32), np.asarray(pwf_w, np.float32),
        np.asarray(pwf_b, np.float32), np.asarray(dwc_w, np.float32),
        np.asarray(dwc_b, np.float32), np.asarray(pwc_w, np.float32),
        np.asarray(pwc_b, np.float32), np.asarray(dwm_w, np.float32),
        np.asarray(dwm_b, np.float32), np.asarray(pwm_w, np.float32),
        np.asarray(pwm_b, np.float32), np.asarray(pc_w, np.float32),
        np.asarray(pc_b, np.float32), np.asarray(mlp_w1, np.float32),
        np.asarray(mlp_b1, np.float32), np.asarray(mlp_w2, np.float32),
        np.asarray(mlp_b2, np.float32),
    )

    in_maps = []
    shards = []
    b2 = stat["b2"][:, 0]
    for core in range(N_CORES):
        bidx, half = divmod(core, 2)
        r0 = half * ROWS_OUT
        shards.append((bidx, r0))
        xp = _pad_img(x[bidx])
        x2 = _build_slab(xp, r0)
        xres = np.zeros((C, NP_OUT), np.float32)
        xres += b2[:, None]
        xres.reshape(C, ROWS_OUT, WP)[:, :, 1 : 1 + W] += \
            x[bidx, :, r0 : r0 + ROWS_OUT, :]
        m = {"wb16": stat["wb16"], "wb8": stat["wb8"],
             "wbf32": stat["wbf32"], "sidx": stat["sidx"],
             "x2": _bf(x2),
             "x8": np.ascontiguousarray(
                 np.asarray(x2, np.float32).astype(ml_dtypes.float8_e4m3fn)),
             "xres": _f32(xres)}
        in_maps.append(m)

    nc = _get_nc()
    kw = dict(_bench) if _bench else {}
    res = run_bass_kernel_spmd(nc, in_maps, list(range(N_CORES)), **kw)

    out = np.zeros((B, OUTC, H, W), np.float32)
    for core, (bidx, r0) in enumerate(shards):
        o = res.results[core]["out"].reshape(OUTC, ROWS_OUT, WP)
        out[bidx, :, r0 : r0 + ROWS_OUT, :] = o[:, :, 1 : 1 + W]
    if _bench is not None:
        _CACHE["last_results"] = res
    return out
